# revision 32
# baseline (speedup 1.0000x reference)
"""Cross-attention MHA (B=2, S=2048, DIM=1024, H=16, DK=64) on 8 trn2 cores.

Sharding: core c -> batch b = c//4, head group g = c%4 (heads 4g..4g+3).
Each core computes its 4 heads' attention + output-projection partial
[S, DIM]; host sums the 4 partials per batch (tensor-parallel unshard).

Device layout (per core), everything transposed so no on-chip transposes
are needed:
  xd = dec_xs[b].T     [DIM, S]   bf16   (d_in on partitions)
  xe = enc_xs[b].T     [DIM, S]   bf16
  mt = mask[b].T       [S_k, S_q] bf16   (0.0 / 1.0)
  wq/wk/wv [DIM, 256]  bf16   columns ordered (head_local, dk); wq pre-scaled 1/sqrt(DK)
  wo [256, DIM]        bf16   rows ordered (head_local, dk)

  QT = wq.T @ xd   [256, S]  (via PSUM, evicted bf16)
  KT = wk.T @ xe   [256, S]
  V  = xe.T @ wv   [S, 256]  stored per k-block with a ones column per head ([128, 4*65])
  per head j, per k-block kb:
    S_T[kb] = KT_j[:, kb].T @ QT_j   [128, S] bf16 psum   (scores transposed)
    P = exp(S_T) * mt[kb]            bf16 sbuf  (one fused DVE mul per head-pair)
    AV += [V_j | 1].T @ P            [65, S] fp32 psum  (row 64 = softmax denom)
  normalize: rb = bcast(denom) on GpSimd; rcp = recip_approx_fast (DVE);
             ATT_j = AV[0:64] * rcp  on GpSimd (off ACT/DVE critical path)
  out_partial = ATT.T @ wo           [S, DIM] fp32 -> DRAM
"""

import sys

sys.path.insert(0, "/opt/trn_rl_repo")

import numpy as np
import ml_dtypes

import concourse.bass as bass
import concourse.mybir as mybir
import concourse.tile as tile
from concourse import bacc
from concourse.bass_utils import run_bass_kernel_spmd

B, S, DIM, H, DK = 2, 2048, 1024, 16, 64
HPC = 4  # heads per core
AD = HPC * DK  # 256 local attention dims
KB = S // 128  # 16 k-blocks
QH = 1024  # q half
BF = mybir.dt.bfloat16
F32 = mybir.dt.float32
bf16 = ml_dtypes.bfloat16


def build_program():
    nc = bacc.Bacc("TRN2", target_bir_lowering=False, debug=False, num_devices=8)

    xd = nc.dram_tensor("xd", [DIM, S], BF, kind="ExternalInput")
    xe = nc.dram_tensor("xe", [DIM, S], BF, kind="ExternalInput")
    mt = nc.dram_tensor("mt", [S, S], BF, kind="ExternalInput")
    wq = nc.dram_tensor("wq", [128, DIM // 128, AD], BF, kind="ExternalInput")
    wk = nc.dram_tensor("wk", [128, DIM // 128, AD], BF, kind="ExternalInput")
    wv = nc.dram_tensor("wv", [128, DIM // 128, AD], BF, kind="ExternalInput")
    wo = nc.dram_tensor("wo", [128, AD // 128, DIM], BF, kind="ExternalInput")
    # bf16 output halves the tail out-DMA; the host upcasts when unsharding.
    out = nc.dram_tensor("out", [S, DIM], BF, kind="ExternalOutput")

    with tile.TileContext(nc) as tc:
        build_tiles(tc, nc, xd, xe, mt, wq, wk, wv, wo, out)

    nc.compile()
    return nc


def build_tiles(tc, nc, xd, xe, mt, wq, wk, wv, wo, out):
    from contextlib import ExitStack

    Exp = mybir.ActivationFunctionType.Exp
    NKC = DIM // 128  # 8 contraction chunks

    with ExitStack() as ctx:
        wpool = ctx.enter_context(tc.tile_pool(name="w", bufs=1))
        qkpool = ctx.enter_context(tc.tile_pool(name="qk", bufs=1))
        vpool = ctx.enter_context(tc.tile_pool(name="v", bufs=1))
        attpool = ctx.enter_context(tc.tile_pool(name="att", bufs=1))
        mpool = ctx.enter_context(tc.tile_pool(name="m", bufs=1))
        xepool = ctx.enter_context(tc.tile_pool(name="xe", bufs=1))
        xdpool = ctx.enter_context(tc.tile_pool(name="xd", bufs=4))
        ppool = ctx.enter_context(tc.tile_pool(name="p", bufs=3))
        avsbpool = ctx.enter_context(tc.tile_pool(name="avsb", bufs=3))
        rbpool = ctx.enter_context(tc.tile_pool(name="rb", bufs=1))
        opool = ctx.enter_context(tc.tile_pool(name="o", bufs=3))

        # ---- weights (pre-arranged [128, c, m] on host; contiguous DMA) ----
        wq_sb = wpool.tile([128, NKC, AD], BF, tag="wq", name="wq_sb")
        wk_sb = wpool.tile([128, NKC, AD], BF, tag="wk", name="wk_sb")
        wv_sb = wpool.tile([128, NKC, AD], BF, tag="wv", name="wv_sb")
        wo_sb = wpool.tile([128, AD // 128, DIM], BF, tag="wo", name="wo_sb")
        # xd chunk 0/1 issued first: the SP issues DMA descriptors serially and
        # the very first matmul needs xd0 + wq.
        xts = {}
        for kc in range(4):
            xt = xdpool.tile([128, S], BF, tag="xd", name="xd_t")
            nc.sync.dma_start(xt[:], xd.ap()[kc * 128 : (kc + 1) * 128, :])
            xts[kc] = xt
        nc.sync.dma_start(wq_sb[:], wq.ap())
        nc.sync.dma_start(wk_sb[:], wk.ap())
        nc.sync.dma_start(wv_sb[:], wv.ap())
        nc.sync.dma_start(wo_sb[:], wo.ap())

        # long-lived activations
        qt_sb = [qkpool.tile([128, S], BF, tag=f"qt{m}", name=f"qt{m}") for m in range(2)]
        kt_sb = [qkpool.tile([128, S], BF, tag=f"kt{m}", name=f"kt{m}") for m in range(2)]
        v_sb = vpool.tile([128, KB, HPC * 65], BF, tag="v", name="v_sb")
        # att split per (m, q-half) so output projection can start per half
        att_q = {
            (m, qh): attpool.tile([128, QH], BF, tag=f"att{m}{qh}", name=f"att{m}{qh}")
            for m in range(2)
            for qh in range(2)
        }
        m_sb = mpool.tile([128, KB, S], BF, tag="mask", name="m_sb")
        xe_sb = xepool.tile([128, NKC, S], BF, tag="xe", name="xe_sb")
        drampool = ctx.enter_context(tc.tile_pool(name="dram", bufs=2, space="DRAM"))

        # ---- phase A: projections ----
        # Q: contraction(kc)-outer with xd streamed; xe prefetch interleaved.
        with tc.tile_pool(name="psA1", bufs=4, space="PSUM") as psA1:
            ps_q = [
                psA1.tile([128, QH], F32, tag="ps_qk", name=f"psq{i}")
                for i in range(4)
            ]
            for kc in range(NKC):
                xt = xts.pop(kc)
                if kc + 4 < NKC:
                    nxt = xdpool.tile([128, S], BF, tag="xd", name="xd_t")
                    nc.sync.dma_start(
                        nxt[:], xd.ap()[(kc + 4) * 128 : (kc + 5) * 128, :]
                    )
                    xts[kc + 4] = nxt
                # issue xe ahead of this kc's matmuls so the K projection's
                # data stream runs a beat earlier than the Q compute.
                nc.sync.dma_start(xe_sb[:, kc, :], xe.ap()[kc * 128 : (kc + 1) * 128, :])
                for m in range(2):
                    for qq in range(2):
                        for nb in range(QH // 512):
                            nc.tensor.matmul(
                                ps_q[m * 2 + qq][:, nb * 512 : (nb + 1) * 512],
                                lhsT=wq_sb[:, kc, m * 128 : (m + 1) * 128],
                                rhs=xt[:, qq * QH + nb * 512 : qq * QH + (nb + 1) * 512],
                                start=(kc == 0),
                                stop=(kc == NKC - 1),
                            )
            for m in range(2):
                for qq in range(2):
                    nc.scalar.copy(
                        qt_sb[m][:, qq * QH : (qq + 1) * QH], ps_q[m * 2 + qq][:]
                    )

        # mask DMAs (own SBUF region; overlap K/V projections)
        for kb in range(KB):
            nc.sync.dma_start(m_sb[:, kb, :], mt.ap()[kb * 128 : (kb + 1) * 128, :])

        # K (mb0) -> V -> K (mb1): attention pair 0 can start after K0+V.
        with tc.tile_pool(name="psA2", bufs=2, space="PSUM") as psA2:
            for m in range(2):
                ps_k = [
                    psA2.tile([128, QH], F32, tag="ps_k", name=f"psk{i}")
                    for i in range(2)
                ]
                for kc in range(NKC):
                    for qq in range(2):
                        for nb in range(QH // 512):
                            nc.tensor.matmul(
                                ps_k[qq][:, nb * 512 : (nb + 1) * 512],
                                lhsT=wk_sb[:, kc, m * 128 : (m + 1) * 128],
                                rhs=xe_sb[
                                    :, kc, qq * QH + nb * 512 : qq * QH + (nb + 1) * 512
                                ],
                                start=(kc == 0),
                                stop=(kc == NKC - 1),
                            )
                for qq in range(2):
                    nc.scalar.copy(
                        kt_sb[m][:, qq * QH : (qq + 1) * QH], ps_k[qq][:]
                    )
                if m == 0:
                    # V: per k-block [128, 256] -> v_sb strided (65-col groups)
                    for kb in range(KB):
                        ps = psA2.tile([128, 256], F32, tag="ps_v", name="ps_v")
                        for kc in range(NKC):
                            nc.tensor.matmul(
                                ps[:],
                                lhsT=xe_sb[:, kc, kb * 128 : (kb + 1) * 128],
                                rhs=wv_sb[:, kc, :],
                                start=(kc == 0),
                                stop=(kc == NKC - 1),
                            )
                        dstv = v_sb[:, kb, :].rearrange("p (j c) -> p j c", c=65)
                        nc.vector.tensor_copy(
                            dstv[:, :, 0:64], ps.rearrange("p (j c) -> p j c", c=64)
                        )
                    ones_dst = v_sb.rearrange("p kb (j c) -> p kb j c", c=65)
                    nc.vector.memset(ones_dst[:, :, :, 64:65], 1.0)

        # ---- phase B: attention ----
        # PSUM: scores s 2x[128,1024] f32 (2-deep ring) = 4 banks
        #       av 2x[128,1024] f32 = 4 banks
        with tc.tile_pool(name="psS", bufs=2, space="PSUM") as psS, tc.tile_pool(
            name="psAV", bufs=2, space="PSUM"
        ) as psAV:
            # One flat (block, kb) stream: the 3-stage software pipeline
            #   stage0 i:   scores -> s psum        (PE)
            #   stage1 i-1: exp (ACT) + fused pair mask-mul (DVE)
            #   stage2 i-2: AV accumulate           (PE)
            # crosses block boundaries, so ACT never drains between blocks.
            blocks = [(qh, pr) for qh in range(S // QH) for pr in range(HPC // 2)]
            stages = [(b, kb) for b in range(len(blocks)) for kb in range(KB)]
            ss_hist = {}
            p_hist = {}
            avs_by_b = {}

            def emit_scores(b, kb):
                qh, pr = blocks[b]
                qsl = slice(qh * QH, (qh + 1) * QH)
                ss = []
                for hh in range(2):
                    s = psS.tile([128, QH], F32, tag="s", name=f"s{hh}")
                    ss.append(s)
                    qt_j = qt_sb[pr][hh * 64 : hh * 64 + 64, qsl]
                    kt_j = kt_sb[pr][hh * 64 : hh * 64 + 64, :]
                    for nb in range(QH // 512):
                        nc.tensor.matmul(
                            s[:, nb * 512 : (nb + 1) * 512],
                            lhsT=kt_j[:, kb * 128 : (kb + 1) * 128],
                            rhs=qt_j[:, nb * 512 : (nb + 1) * 512],
                            start=True,
                            stop=True,
                        )
                ss_hist[(b, kb)] = ss

            def emit_exp(b, kb):
                qh, pr = blocks[b]
                qsl = slice(qh * QH, (qh + 1) * QH)
                ss = ss_hist.pop((b, kb))
                pp = ppool.tile([128, 2 * QH], BF, tag="p", name="pp")
                for hh in range(2):
                    nc.scalar.activation(
                        pp[:, hh * QH : (hh + 1) * QH], ss[hh][:], Exp
                    )
                # one fused mask multiply for the pair: mask broadcast
                # across the head dim via a 0-stride AP.
                ppv = pp.rearrange("p (h q) -> p h q", h=2)
                mb = m_sb[:, kb, qsl].unsqueeze(1).broadcast_to([128, 2, QH])
                nc.vector.tensor_mul(ppv, ppv, mb)
                p_hist[(b, kb)] = pp

            def emit_av(b, kb):
                qh, pr = blocks[b]
                if kb == 0:
                    avs_by_b[b] = [
                        psAV.tile([128, QH], F32, tag="av", name=f"av{hh}")
                        for hh in range(2)
                    ]
                avs = avs_by_b[b]
                pp = p_hist.pop((b, kb))
                for hh in range(2):
                    j = 2 * pr + hh
                    for nb in range(QH // 512):
                        nc.tensor.matmul(
                            avs[hh][0:65, nb * 512 : (nb + 1) * 512],
                            lhsT=v_sb[:, kb, j * 65 : (j + 1) * 65],
                            rhs=pp[:, hh * QH + nb * 512 : hh * QH + (nb + 1) * 512],
                            start=(kb == 0),
                            stop=(kb == KB - 1),
                        )
                if kb == KB - 1:
                    normalize(b)

            def normalize(b):
                qh, pr = blocks[b]
                avs = avs_by_b.pop(b)
                # evict av (vals + denom row) to SBUF, freeing PSUM fast.
                avts = {}
                for hh in range(2):
                    avt = avsbpool.tile([65, QH], F32, tag="avsb", name=f"avt{hh}")
                    nc.vector.tensor_copy(avt[:], avs[hh][0:65, :])
                    avts[hh] = avt
                # normalize off the critical path: denom rows bounce once
                # through DRAM for the partition broadcast, then one fast
                # pair-wide reciprocal (DVE) and two DVE multiplies.
                den_scr = drampool.tile([2, QH], F32, tag="den", name="den_scr")
                for hh in range(2):
                    nc.sync.dma_start(den_scr[hh : hh + 1, :], avts[hh][64:65, :])
                rbp = rbpool.tile([64, 2 * QH], F32, tag="rb", name="rbp")
                for hh in range(2):
                    nc.sync.dma_start(
                        rbp[:, hh * QH : (hh + 1) * QH],
                        den_scr[hh, :].partition_broadcast(64),
                    )
                rcp = rbpool.tile([64, 2 * QH], F32, tag="rcp", name="rcp")
                nc.vector.reciprocal_approx_fast(rcp[:], rbp[:])
                for hh in range(2):
                    att_dst = att_q[(pr, qh)][hh * 64 : hh * 64 + 64, :]
                    nc.vector.tensor_mul(
                        att_dst, avts[hh][0:64, :], rcp[:, hh * QH : (hh + 1) * QH]
                    )

            for i in range(len(stages)):
                emit_scores(*stages[i])
                if i >= 1:
                    emit_exp(*stages[i - 1])
                if i >= 2:
                    emit_av(*stages[i - 2])
            emit_exp(*stages[-1])
            emit_av(*stages[-2])
            emit_av(*stages[-1])

        # ---- phase C: output projection ----
        with tc.tile_pool(name="psO", bufs=3, space="PSUM") as psO:
            for qb in range(S // 128):
                qh, i = qb // (QH // 128), qb % (QH // 128)
                po = psO.tile([128, DIM], F32, tag="po", name="po")
                for cc in range(AD // 128):
                    for nb in range(DIM // 512):
                        nc.tensor.matmul(
                            po[:, nb * 512 : (nb + 1) * 512],
                            lhsT=att_q[(cc, qh)][:, i * 128 : (i + 1) * 128],
                            rhs=wo_sb[:, cc, nb * 512 : (nb + 1) * 512],
                            start=(cc == 0),
                            stop=(cc == AD // 128 - 1),
                        )
                ob = opool.tile([128, DIM], BF, tag="ob", name="ob")
                if qb % 2 == 0:
                    nc.vector.tensor_copy(ob[:], po[:])
                else:
                    nc.scalar.copy(ob[:], po[:])
                nc.sync.dma_start(out.ap()[qb * 128 : (qb + 1) * 128, :], ob[:])


def make_core_inputs(dec_xs, enc_xs, Wq, Wkv, Wo, mask):
    """Host-side sharding: returns list of 8 in_maps."""
    dec_xs = np.asarray(dec_xs, dtype=np.float32)
    enc_xs = np.asarray(enc_xs, dtype=np.float32)
    Wq = np.asarray(Wq, dtype=np.float32)
    Wkv = np.asarray(Wkv, dtype=np.float32)
    Wo = np.asarray(Wo, dtype=np.float32)
    mask = np.asarray(mask)

    Wk = Wkv[:DIM]
    Wv = Wkv[DIM:]

    xds, xes, mts = [], [], []
    for b in range(B):
        xds.append(np.ascontiguousarray(dec_xs[b].T).astype(bf16))
        xes.append(np.ascontiguousarray(enc_xs[b].T).astype(bf16))
        mts.append(np.ascontiguousarray(mask[b].T).astype(bf16))

    in_maps = []
    for c in range(8):
        b, g = divmod(c, 4)
        # local att col (j*64 + dk) <- global feature dk*H + (4g + j)
        hsel = np.array(
            [dk * H + (4 * g + j) for j in range(HPC) for dk in range(DK)],
            dtype=np.int64,
        )
        def arrange(w2d):
            # [D_in, M] -> [128, D_in//128, M] partition-major chunks
            d, mcols = w2d.shape
            return np.ascontiguousarray(
                w2d.reshape(d // 128, 128, mcols).transpose(1, 0, 2)
            ).astype(bf16)

        wq_l = arrange((Wq[hsel, :] / np.sqrt(DK)).T)
        wk_l = arrange(Wk[hsel, :].T)
        wv_l = arrange(Wv[hsel, :].T)
        wo_l = arrange(Wo[:, hsel].T)
        in_maps.append(
            {
                "xd": xds[b],
                "xe": xes[b],
                "mt": mts[b],
                "wq": wq_l,
                "wk": wk_l,
                "wv": wv_l,
                "wo": wo_l,
            }
        )
    return in_maps


_NC = None


def _get_nc():
    global _NC
    if _NC is None:
        _NC = build_program()
    return _NC


def kernel(dec_xs, enc_xs, Wq, Wkv, Wo, mask):
    nc = _get_nc()
    in_maps = make_core_inputs(dec_xs, enc_xs, Wq, Wkv, Wo, mask)
    res = run_bass_kernel_spmd(nc, in_maps, list(range(8)))
    out = np.zeros((B, S, DIM), np.float32)
    for c in range(8):
        out[c // 4] += np.asarray(res.results[c]["out"], dtype=np.float32)
    return out
